# revision 33
# baseline (speedup 1.0000x reference)
"""Multi-head attention (B=4, L=2048, D=1024, H=16) on 8 Trainium2 NeuronCores.

Sharding: tensor-parallel over heads — core c owns heads {2c, 2c+1}.
Each core computes Q/K/V projections for its 128 feature dims, full
attention for its 2 heads over all 4 batches, and a partial final
projection y_c = attn_out_c @ Wf[:, 128c:128c+128].T.  The host sums the
8 partials (plus the bias constant Wf@bv + bf, which commutes out since
softmax rows sum to 1).

Key compaction: masked keys contribute exactly 0 to softmax(masked
logits), so the host gathers only the unmasked key rows (padded to a
multiple of 128, zero-weighted via m') and K/V projections + QK + exp +
PV run on the compacted key set only (~56% of L for a p=0.5 mask).

Device layouts (per core):
  qT     [128(2h x 64d), L]   bf16   (feature dims in partitions)
  kT     [128, LC]            bf16   (compacted keys)
  v'     [128(j rows), NJT, 130] bf16: per j-row-tile [v_h0*m | m | v_h1*m | m]
         where m = 1 for real keys, 0 for padding (this implements both
         masking and padding: exp(logit)*m / sum_j exp*m)
  logitsT tiles [128 j, 512 i] in PSUM -> exp on ScalarE -> bf16 SBUF
  PV: out'[65, 512] = v'.T @ expT accumulated over NJT j-tiles; row 64 is
      the softmax denominator (ones-column trick).
  normalize: recip_approx(row64), gpsimd partition-broadcast, DVE multiply.
  final: y[i,fo] = outT.T @ WfcT, K=128 (both heads, already normalized).

Emission is software-pipelined across batches (attention(b) interleaved
with projections(b+1) and final(b)) so the PE always has ready matmuls
and the HAM clock gate stays at full rate.
"""

import numpy as np
import ml_dtypes
from contextlib import ExitStack

import concourse.bass as bass
import concourse.tile as tile
from concourse import bacc, mybir
from concourse import masks
from concourse.bass_utils import run_bass_kernel_spmd

B, L, D, H, DK = 4, 2048, 1024, 16, 64
R = B * L                      # 8192 rows
NCORES = 8
HPC = H // NCORES              # 2 heads per core
DC = HPC * DK                  # 128 feature dims per core
KCH = D // 128                 # 8 contraction chunks
SCALE = 1.0 / np.sqrt(DK)

BF16 = mybir.dt.bfloat16
F32 = mybir.dt.float32
EXP = mybir.ActivationFunctionType.Exp

TRACE = False                  # test.py flips this for profiling runs
LAST_RESULT = {}               # exec_time_ns etc. stashed here for test.py


def build_bass(njt):
    """njt = number of 128-key tiles per batch after compaction."""
    lc = njt * 128             # compacted keys per batch
    kv_widths = [512] * (lc // 512) + ([lc % 512] if lc % 512 else [])

    nc = bacc.Bacc(
        "TRN2",
        target_bir_lowering=False,
        debug=False,
        enable_asserts=False,
        num_devices=NCORES,
    )
    xt = nc.dram_tensor("xt", [D, R], BF16, kind="ExternalInput").ap()
    xtc = nc.dram_tensor("xtc", [D, B * lc], BF16, kind="ExternalInput").ap()
    wq = nc.dram_tensor("wq", [KCH, 128, DC], BF16, kind="ExternalInput").ap()
    wk = nc.dram_tensor("wk", [KCH, 128, DC], BF16, kind="ExternalInput").ap()
    wv = nc.dram_tensor("wv", [KCH, 128, DC], BF16, kind="ExternalInput").ap()
    wf = nc.dram_tensor("wf", [DC, D], BF16, kind="ExternalInput").ap()
    bq = nc.dram_tensor("bq", [128, 1], F32, kind="ExternalInput").ap()
    bk = nc.dram_tensor("bk", [128, 1], F32, kind="ExternalInput").ap()
    mp = nc.dram_tensor("mp", [128, B * njt], F32, kind="ExternalInput").ap()
    y = nc.dram_tensor("y", [R, D], F32, kind="ExternalOutput").ap()

    with tile.TileContext(nc) as tc, ExitStack() as ctx:
        singles = ctx.enter_context(tc.tile_pool(name="singles", bufs=1))
        qkpool = ctx.enter_context(tc.tile_pool(name="qk", bufs=3))
        xin = ctx.enter_context(tc.tile_pool(name="xin", bufs=6))
        expp = ctx.enter_context(tc.tile_pool(name="expp", bufs=6))
        pvsb = ctx.enter_context(tc.tile_pool(name="pvsb", bufs=3))
        ysb = ctx.enter_context(tc.tile_pool(name="ysb", bufs=3))
        ps_small = ctx.enter_context(tc.tile_pool(name="ps_small", bufs=2, space="PSUM"))
        ps_pair = ctx.enter_context(tc.tile_pool(name="ps_pair", bufs=2, space="PSUM"))
        ps_pv = ctx.enter_context(tc.tile_pool(name="ps_pv", bufs=2, space="PSUM"))

        wq_sb = singles.tile([128, KCH, DC], BF16)
        nc.sync.dma_start(wq_sb, wq.rearrange("c p f -> p c f"))
        wk_sb = singles.tile([128, KCH, DC], BF16)
        nc.sync.dma_start(wk_sb, wk.rearrange("c p f -> p c f"))
        wv_sb = singles.tile([128, KCH, DC], BF16)
        nc.sync.dma_start(wv_sb, wv.rearrange("c p f -> p c f"))
        wf_sb = singles.tile([DC, D], BF16)
        nc.sync.dma_start(wf_sb, wf)
        bq_sb = singles.tile([128, 1], F32)
        nc.sync.dma_start(bq_sb, bq)
        bk_sb = singles.tile([128, 1], F32)
        nc.sync.dma_start(bk_sb, bk)
        mp_sb = singles.tile([128, B * njt], F32)
        nc.sync.dma_start(mp_sb, mp)
        ident_sb = singles.tile([128, 128], BF16)
        masks.make_identity(nc, ident_sb)

        xt_r = xt.rearrange("(c p) r -> p c r", p=128)
        xtc_r = xtc.rearrange("(c p) r -> p c r", p=128)

        bt = {}  # per-batch persistent tiles

        def tiles(b):
            if b not in bt:
                bt[b] = (
                    qkpool.tile([128, L], BF16, tag="qT", name=f"qT{b}"),
                    qkpool.tile([128, lc], BF16, tag="kT", name=f"kT{b}"),
                    qkpool.tile([128, njt, 2 * (DK + 1)], BF16, tag="vb",
                                name=f"vb{b}"),
                    qkpool.tile([128, L], BF16, tag="oT", name=f"oT{b}"),
                )
            return bt[b]

        def phase1_q(b, blk):
            """Q projection for rows [b*L + blk*512, +512)."""
            qT, _, _, _ = tiles(b)
            r0 = b * L + blk * 512
            xt_t = xin.tile([128, KCH, 512], BF16, tag="xt", name=f"xt{b}_{blk}")
            nc.sync.dma_start(xt_t, xt_r[:, :, r0:r0 + 512])
            ps = ps_small.tile([128, 512], F32, tag="pss", name=f"psq{b}_{blk}")
            for k in range(KCH):
                nc.tensor.matmul(ps, wq_sb[:, k, :], xt_t[:, k, :],
                                 start=(k == 0), stop=(k == KCH - 1))
            nc.vector.tensor_scalar_add(
                qT[:, blk * 512:(blk + 1) * 512], ps, bq_sb)

        def phase1_kv(b, blk):
            """K + V projections for compacted keys [blk*512, +w) of batch b."""
            _, kT, vb, _ = tiles(b)
            w = kv_widths[blk]
            c0 = blk * 512
            xc_t = xin.tile([128, KCH, 512], BF16, tag="xc", name=f"xc{b}_{blk}")
            nc.sync.dma_start(xc_t[:, :, 0:w],
                              xtc_r[:, :, b * lc + c0: b * lc + c0 + w])
            ps = ps_small.tile([128, 512], F32, tag="pss", name=f"psk{b}_{blk}")
            for k in range(KCH):
                nc.tensor.matmul(ps[:, 0:w], wk_sb[:, k, :], xc_t[:, k, 0:w],
                                 start=(k == 0), stop=(k == KCH - 1))
            nc.vector.tensor_scalar_add(kT[:, c0:c0 + w], ps[:, 0:w], bk_sb)
            # vT block [d 128, r w] then PE-transpose into [r, d] v' tiles
            psvT = ps_small.tile([128, 512], F32, tag="pss", name=f"psvT{b}_{blk}")
            for k in range(KCH):
                nc.tensor.matmul(psvT[:, 0:w], wv_sb[:, k, :], xc_t[:, k, 0:w],
                                 start=(k == 0), stop=(k == KCH - 1))
            vt_sb = xin.tile([128, 512], BF16, tag="vt", name=f"vt{b}_{blk}")
            nc.vector.tensor_copy(vt_sb[:, 0:w], psvT[:, 0:w])
            for rs in range(w // 128):
                t = blk * 4 + rs            # j-tile within batch
                tg = b * njt + t            # global compacted row-tile
                pst = ps_small.tile([128, 128], BF16, tag="pss",
                                    name=f"pst{b}_{blk}_{rs}")
                nc.tensor.transpose(pst, vt_sb[:, rs * 128:(rs + 1) * 128],
                                    ident_sb)
                mcol = mp_sb[:, tg:tg + 1]
                nc.vector.tensor_scalar_mul(vb[:, t, 0:DK], pst[:, 0:DK], mcol)
                nc.vector.tensor_scalar_mul(
                    vb[:, t, DK + 1:2 * DK + 1], pst[:, DK:DC], mcol)
                nc.vector.tensor_copy(vb[:, t, DK:DK + 1], mcol)
                nc.vector.tensor_copy(vb[:, t, 2 * DK + 1:2 * DK + 2], mcol)

        def phase1_block(b, blk):
            phase1_q(b, blk)
            if blk < len(kv_widths):
                phase1_kv(b, blk)

        def attn_it(b, it, fillers=()):
            """Attention for queries [b*L + it*512, +512), both heads."""
            fillers = list(fillers)
            qT, kT, vb, oT = tiles(b)
            i0 = it * 512
            pvs = [ps_pv.tile([DK + 1, 512], F32, tag="pv", name=f"pv{b}_{it}_{h}")
                   for h in range(HPC)]
            for jp in range((njt + 1) // 2):
                if fillers:
                    fillers.pop(0)()
                ns = min(2, njt - jp * 2)
                for h in range(HPC):
                    ps = ps_pair.tile([128, 2, 512], F32, tag="pair",
                                      name=f"qk{b}_{it}_{jp}_{h}")
                    for s in range(ns):
                        jt = jp * 2 + s
                        nc.tensor.matmul(
                            ps[:, s, :],
                            kT[h * DK:(h + 1) * DK, jt * 128:(jt + 1) * 128],
                            qT[h * DK:(h + 1) * DK, i0:i0 + 512],
                            start=True, stop=True,
                            tile_position=(h * DK, 0))
                    ex = expp.tile([128, 2, 512], BF16, tag="ex",
                                   name=f"ex{b}_{it}_{jp}_{h}")
                    nc.scalar.activation(ex[:, 0:ns, :], ps[:, 0:ns, :], EXP)
                    for s in range(ns):
                        jt = jp * 2 + s
                        nc.tensor.matmul(
                            pvs[h],
                            vb[:, jt, h * (DK + 1):(h + 1) * (DK + 1)],
                            ex[:, s, :],
                            start=(jt == 0), stop=(jt == njt - 1))
            # normalize: rrow = 1/sums (both heads batched in one partition),
            # broadcast via gpsimd, multiply.
            srow = pvsb.tile([1, HPC, 512], F32, tag="srow", name=f"sr{b}_{it}")
            for h in range(HPC):
                nc.vector.tensor_copy(srow[0:1, h, :], pvs[h][DK:DK + 1, :])
            rrow = pvsb.tile([1, HPC, 512], F32, tag="rrow", name=f"rr{b}_{it}")
            nc.vector.reciprocal_approx_fast(rrow, srow)
            for h in range(HPC):
                rb_sb = pvsb.tile([DK, 512], F32, tag="rb", name=f"rb{b}_{it}_{h}")
                nc.gpsimd.partition_broadcast(rb_sb, rrow[0:1, h, :])
                nc.vector.tensor_mul(
                    oT[h * DK:(h + 1) * DK, i0:i0 + 512], pvs[h][0:DK, :], rb_sb)

        def final_yt(b, i2):
            """Final projection for one 128-row i-tile of batch b."""
            _, _, _, oT = tiles(b)
            yt = ysb.tile([128, D], F32, tag="y", name=f"y{b}_{i2}")
            for fo in range(D // 512):
                psf = ps_small.tile([128, 512], F32, tag="pss",
                                    name=f"psf{b}_{i2}_{fo}")
                nc.tensor.matmul(psf, oT[:, i2 * 128:(i2 + 1) * 128],
                                 wf_sb[:, fo * 512:(fo + 1) * 512],
                                 start=True, stop=True)
                # split evacuation between ScalarE and VectorE
                if fo == 0:
                    nc.scalar.copy(yt[:, fo * 512:(fo + 1) * 512], psf)
                else:
                    nc.vector.tensor_copy(yt[:, fo * 512:(fo + 1) * 512], psf)
            nc.sync.dma_start(
                y[b * L + i2 * 128: b * L + (i2 + 1) * 128, :], yt)

        # software-pipelined emission: attention(b) | projections(b+1) | final(b)
        # prologue: only what attention(0,0) needs — kv blocks + first q block
        for blk in range(len(kv_widths)):
            phase1_kv(0, blk)
        phase1_q(0, 0)
        for b in range(B):
            for it in range(4):
                if b == 0 and it + 1 < 4:
                    phase1_q(0, it + 1)
                attn_it(b, it)
                if b + 1 < B:
                    phase1_block(b + 1, it)
                for i2 in range(it * 4, it * 4 + 4):
                    final_yt(b, i2)

    nc.compile()
    return nc


def make_in_maps(x, mask, Wq, bq, Wk, bk, Wv, bv, Wf, bf, njt):
    bf16 = ml_dtypes.bfloat16
    lc = njt * 128
    xt = np.ascontiguousarray(x.reshape(R, D).T).astype(bf16)

    # compacted key columns + their validity weights
    cols = np.zeros(B * lc, np.int64)
    mprime = np.zeros(B * lc, np.float32)
    m = np.asarray(mask)
    for b in range(B):
        idx = np.nonzero(m[b] == 0)[0] + b * L
        cols[b * lc: b * lc + len(idx)] = idx
        mprime[b * lc: b * lc + len(idx)] = 1.0
    xtc = np.ascontiguousarray(xt[:, cols])
    mp = np.ascontiguousarray(mprime.reshape(B * njt, 128).T).astype(np.float32)

    in_maps = []
    for c in range(NCORES):
        sl = slice(c * DC, (c + 1) * DC)
        wq_c = np.ascontiguousarray((Wq[sl, :] * SCALE).T).astype(bf16).reshape(KCH, 128, DC)
        wk_c = np.ascontiguousarray(Wk[sl, :].T).astype(bf16).reshape(KCH, 128, DC)
        wv_c = np.ascontiguousarray(Wv[sl, :].T).astype(bf16).reshape(KCH, 128, DC)
        wf_c = np.ascontiguousarray(Wf[:, sl].T).astype(bf16)
        bq_c = (bq[sl] * SCALE).astype(np.float32).reshape(128, 1)
        bk_c = bk[sl].astype(np.float32).reshape(128, 1)
        in_maps.append({
            "xt": xt, "xtc": xtc, "wq": wq_c, "wk": wk_c, "wv": wv_c,
            "wf": wf_c, "bq": bq_c, "bk": bk_c, "mp": mp,
        })
    return in_maps


_NC_CACHE = {}


def kernel(**inputs):
    x = np.asarray(inputs["x"], np.float32)
    mask = np.asarray(inputs["mask"])
    Wq = np.asarray(inputs["Wq"], np.float32)
    bq = np.asarray(inputs["bq"], np.float32)
    Wk = np.asarray(inputs["Wk"], np.float32)
    bk = np.asarray(inputs["bk"], np.float32)
    Wv = np.asarray(inputs["Wv"], np.float32)
    bv = np.asarray(inputs["bv"], np.float32)
    Wf = np.asarray(inputs["Wf"], np.float32)
    bf = np.asarray(inputs["bf"], np.float32)

    nmax = int((mask == 0).sum(axis=1).max())
    nmax = max(nmax, 1)
    njt = (nmax + 127) // 128

    in_maps = make_in_maps(x, mask, Wq, bq, Wk, bk, Wv, bv, Wf, bf, njt)
    if njt not in _NC_CACHE:
        _NC_CACHE[njt] = build_bass(njt)
    nc = _NC_CACHE[njt]

    res = run_bass_kernel_spmd(nc, in_maps, core_ids=list(range(NCORES)),
                               trace=TRACE)
    LAST_RESULT["exec_time_ns"] = res.exec_time_ns
    LAST_RESULT["trace"] = res.instructions_and_trace

    y = res.results[0]["y"].astype(np.float32).copy()
    for c in range(1, NCORES):
        y += res.results[c]["y"]
    # bv/bf fold: softmax rows sum to 1, so attn_out bias bv contributes
    # Wf @ bv; bf is the plain output bias.
    y += (Wf @ bv + bf)[None, :]
    return y.reshape(B, L, D).astype(np.float32)


# revision 34
# speedup vs baseline: 1.0549x; 1.0549x over previous
"""Multi-head attention (B=4, L=2048, D=1024, H=16) on 8 Trainium2 NeuronCores.

Sharding: tensor-parallel over heads — core c owns heads {2c, 2c+1}.
Each core computes Q/K/V projections for its 128 feature dims, full
attention for its 2 heads over all 4 batches, and a partial final
projection y_c = attn_out_c @ Wf[:, 128c:128c+128].T.  The host sums the
8 partials (plus the bias constant Wf@bv + bf, which commutes out since
softmax rows sum to 1).

Key compaction: masked keys contribute exactly 0 to softmax(masked
logits), so the host gathers only the unmasked key rows (padded to a
multiple of 128, zero-weighted via m') and K/V projections + QK + exp +
PV run on the compacted key set only (~56% of L for a p=0.5 mask).

Device layouts (per core):
  qT     [128(2h x 64d), L]   bf16   (feature dims in partitions)
  kT     [128, LC]            bf16   (compacted keys)
  v'     [128(j rows), NJT, 130] bf16: per j-row-tile [v_h0*m | m | v_h1*m | m]
         where m = 1 for real keys, 0 for padding (this implements both
         masking and padding: exp(logit)*m / sum_j exp*m)
  logitsT tiles [128 j, 512 i] in PSUM -> exp on ScalarE -> bf16 SBUF
  PV: out'[65, 512] = v'.T @ expT accumulated over NJT j-tiles; row 64 is
      the softmax denominator (ones-column trick).
  normalize: recip_approx(row64), gpsimd partition-broadcast, DVE multiply.
  final: y[i,fo] = outT.T @ WfcT, K=128 (both heads, already normalized).

Emission is software-pipelined across batches (attention(b) interleaved
with projections(b+1) and final(b)) so the PE always has ready matmuls
and the HAM clock gate stays at full rate.
"""

import numpy as np
import ml_dtypes
from contextlib import ExitStack

import concourse.bass as bass
import concourse.tile as tile
from concourse import bacc, mybir
from concourse import masks
from concourse.bass_utils import run_bass_kernel_spmd

B, L, D, H, DK = 4, 2048, 1024, 16, 64
R = B * L                      # 8192 rows
NCORES = 8
HPC = H // NCORES              # 2 heads per core
DC = HPC * DK                  # 128 feature dims per core
KCH = D // 128                 # 8 contraction chunks
SCALE = 1.0 / np.sqrt(DK)

BF16 = mybir.dt.bfloat16
F32 = mybir.dt.float32
EXP = mybir.ActivationFunctionType.Exp

TRACE = False                  # test.py flips this for profiling runs
LAST_RESULT = {}               # exec_time_ns etc. stashed here for test.py


def build_bass(njt):
    """njt = number of 128-key tiles per batch after compaction."""
    lc = njt * 128             # compacted keys per batch
    kv_widths = [512] * (lc // 512) + ([lc % 512] if lc % 512 else [])

    nc = bacc.Bacc(
        "TRN2",
        target_bir_lowering=False,
        debug=False,
        enable_asserts=False,
        num_devices=NCORES,
    )
    xt = nc.dram_tensor("xt", [D, R], BF16, kind="ExternalInput").ap()
    xtc = nc.dram_tensor("xtc", [D, B * lc], BF16, kind="ExternalInput").ap()
    wq = nc.dram_tensor("wq", [KCH, 128, DC], BF16, kind="ExternalInput").ap()
    wk = nc.dram_tensor("wk", [KCH, 128, DC], BF16, kind="ExternalInput").ap()
    wv = nc.dram_tensor("wv", [KCH, 128, DC], BF16, kind="ExternalInput").ap()
    wf = nc.dram_tensor("wf", [DC, D], BF16, kind="ExternalInput").ap()
    bq = nc.dram_tensor("bq", [128, 1], F32, kind="ExternalInput").ap()
    bk = nc.dram_tensor("bk", [128, 1], F32, kind="ExternalInput").ap()
    mp = nc.dram_tensor("mp", [128, B * njt], F32, kind="ExternalInput").ap()
    y = nc.dram_tensor("y", [R, D], F32, kind="ExternalOutput").ap()

    with tile.TileContext(nc) as tc, ExitStack() as ctx:
        singles = ctx.enter_context(tc.tile_pool(name="singles", bufs=1))
        qkpool = ctx.enter_context(tc.tile_pool(name="qk", bufs=3))
        xin = ctx.enter_context(tc.tile_pool(name="xin", bufs=6))
        expp = ctx.enter_context(tc.tile_pool(name="expp", bufs=6))
        pvsb = ctx.enter_context(tc.tile_pool(name="pvsb", bufs=3))
        ysb = ctx.enter_context(tc.tile_pool(name="ysb", bufs=3))
        ps_small = ctx.enter_context(tc.tile_pool(name="ps_small", bufs=2, space="PSUM"))
        ps_pair = ctx.enter_context(tc.tile_pool(name="ps_pair", bufs=2, space="PSUM"))
        ps_pv = ctx.enter_context(tc.tile_pool(name="ps_pv", bufs=2, space="PSUM"))

        wq_sb = singles.tile([128, KCH, DC], BF16)
        nc.sync.dma_start(wq_sb, wq.rearrange("c p f -> p c f"))
        wk_sb = singles.tile([128, KCH, DC], BF16)
        nc.sync.dma_start(wk_sb, wk.rearrange("c p f -> p c f"))
        wv_sb = singles.tile([128, KCH, DC], BF16)
        nc.sync.dma_start(wv_sb, wv.rearrange("c p f -> p c f"))
        wf_sb = singles.tile([DC, D], BF16)
        nc.sync.dma_start(wf_sb, wf)
        bq_sb = singles.tile([128, 1], F32)
        nc.sync.dma_start(bq_sb, bq)
        bk_sb = singles.tile([128, 1], F32)
        nc.sync.dma_start(bk_sb, bk)
        mp_sb = singles.tile([128, B * njt], F32)
        nc.sync.dma_start(mp_sb, mp)
        ident_sb = singles.tile([128, 128], BF16)
        masks.make_identity(nc, ident_sb)

        xt_r = xt.rearrange("(c p) r -> p c r", p=128)
        xtc_r = xtc.rearrange("(c p) r -> p c r", p=128)

        bt = {}  # per-batch persistent tiles

        def tiles(b):
            if b not in bt:
                bt[b] = (
                    qkpool.tile([128, L], BF16, tag="qT", name=f"qT{b}"),
                    qkpool.tile([128, lc], BF16, tag="kT", name=f"kT{b}"),
                    qkpool.tile([128, njt, 2 * (DK + 1)], BF16, tag="vb",
                                name=f"vb{b}"),
                    qkpool.tile([128, L], BF16, tag="oT", name=f"oT{b}"),
                )
            return bt[b]

        def phase1_q(b, blk):
            """Q projection for rows [b*L + blk*512, +512)."""
            qT, _, _, _ = tiles(b)
            r0 = b * L + blk * 512
            xt_t = xin.tile([128, KCH, 512], BF16, tag="xt", name=f"xt{b}_{blk}")
            nc.sync.dma_start(xt_t, xt_r[:, :, r0:r0 + 512])
            ps = ps_small.tile([128, 512], F32, tag="pss", name=f"psq{b}_{blk}")
            for k in range(KCH):
                nc.tensor.matmul(ps, wq_sb[:, k, :], xt_t[:, k, :],
                                 start=(k == 0), stop=(k == KCH - 1))
            nc.vector.tensor_scalar_add(
                qT[:, blk * 512:(blk + 1) * 512], ps, bq_sb)

        def phase1_kv(b, blk):
            """K + V projections for compacted keys [blk*512, +w) of batch b."""
            _, kT, vb, _ = tiles(b)
            w = kv_widths[blk]
            c0 = blk * 512
            xc_t = xin.tile([128, KCH, 512], BF16, tag="xc", name=f"xc{b}_{blk}")
            nc.sync.dma_start(xc_t[:, :, 0:w],
                              xtc_r[:, :, b * lc + c0: b * lc + c0 + w])
            ps = ps_small.tile([128, 512], F32, tag="pss", name=f"psk{b}_{blk}")
            for k in range(KCH):
                nc.tensor.matmul(ps[:, 0:w], wk_sb[:, k, :], xc_t[:, k, 0:w],
                                 start=(k == 0), stop=(k == KCH - 1))
            nc.vector.tensor_scalar_add(kT[:, c0:c0 + w], ps[:, 0:w], bk_sb)
            # vT block [d 128, r w] then PE-transpose into [r, d] v' tiles
            psvT = ps_small.tile([128, 512], F32, tag="pss", name=f"psvT{b}_{blk}")
            for k in range(KCH):
                nc.tensor.matmul(psvT[:, 0:w], wv_sb[:, k, :], xc_t[:, k, 0:w],
                                 start=(k == 0), stop=(k == KCH - 1))
            vt_sb = xin.tile([128, 512], BF16, tag="vt", name=f"vt{b}_{blk}")
            nc.vector.tensor_copy(vt_sb[:, 0:w], psvT[:, 0:w])
            for rs in range(w // 128):
                t = blk * 4 + rs            # j-tile within batch
                tg = b * njt + t            # global compacted row-tile
                pst = ps_small.tile([128, 128], BF16, tag="pss",
                                    name=f"pst{b}_{blk}_{rs}")
                nc.tensor.transpose(pst, vt_sb[:, rs * 128:(rs + 1) * 128],
                                    ident_sb)
                mcol = mp_sb[:, tg:tg + 1]
                nc.vector.tensor_scalar_mul(vb[:, t, 0:DK], pst[:, 0:DK], mcol)
                nc.vector.tensor_scalar_mul(
                    vb[:, t, DK + 1:2 * DK + 1], pst[:, DK:DC], mcol)
                nc.vector.tensor_copy(vb[:, t, DK:DK + 1], mcol)
                nc.vector.tensor_copy(vb[:, t, 2 * DK + 1:2 * DK + 2], mcol)

        def phase1_block(b, blk):
            phase1_q(b, blk)
            if blk < len(kv_widths):
                phase1_kv(b, blk)

        def attn_it(b, it, fillers=()):
            """Attention for queries [b*L + it*512, +512), both heads."""
            fillers = list(fillers)
            qT, kT, vb, oT = tiles(b)
            i0 = it * 512
            pvs = [ps_pv.tile([DK + 1, 512], F32, tag="pv", name=f"pv{b}_{it}_{h}")
                   for h in range(HPC)]
            for jp in range((njt + 1) // 2):
                if fillers:
                    fillers.pop(0)()
                ns = min(2, njt - jp * 2)
                for h in range(HPC):
                    ps = ps_pair.tile([128, 2, 512], F32, tag="pair",
                                      name=f"qk{b}_{it}_{jp}_{h}")
                    for s in range(ns):
                        jt = jp * 2 + s
                        nc.tensor.matmul(
                            ps[:, s, :],
                            kT[h * DK:(h + 1) * DK, jt * 128:(jt + 1) * 128],
                            qT[h * DK:(h + 1) * DK, i0:i0 + 512],
                            start=True, stop=True,
                            tile_position=(h * DK, 0))
                    ex = expp.tile([128, 2, 512], BF16, tag="ex",
                                   name=f"ex{b}_{it}_{jp}_{h}")
                    nc.scalar.activation(ex[:, 0:ns, :], ps[:, 0:ns, :], EXP)
                    for s in range(ns):
                        jt = jp * 2 + s
                        nc.tensor.matmul(
                            pvs[h],
                            vb[:, jt, h * (DK + 1):(h + 1) * (DK + 1)],
                            ex[:, s, :],
                            start=(jt == 0), stop=(jt == njt - 1))
            # normalize: rrow = 1/sums (both heads batched in one partition),
            # broadcast via gpsimd, multiply.
            srow = pvsb.tile([1, HPC, 512], F32, tag="srow", name=f"sr{b}_{it}")
            for h in range(HPC):
                nc.vector.tensor_copy(srow[0:1, h, :], pvs[h][DK:DK + 1, :])
            rrow = pvsb.tile([1, HPC, 512], F32, tag="rrow", name=f"rr{b}_{it}")
            nc.vector.reciprocal_approx_fast(rrow, srow)
            for h in range(HPC):
                rb_sb = pvsb.tile([DK, 512], F32, tag="rb", name=f"rb{b}_{it}_{h}")
                nc.gpsimd.partition_broadcast(rb_sb, rrow[0:1, h, :])
                nc.vector.tensor_mul(
                    oT[h * DK:(h + 1) * DK, i0:i0 + 512], pvs[h][0:DK, :], rb_sb)

        def final_yt(b, i2):
            """Final projection for one 128-row i-tile of batch b."""
            _, _, _, oT = tiles(b)
            yt = ysb.tile([128, D], F32, tag="y", name=f"y{b}_{i2}")
            for fo in range(D // 512):
                psf = ps_small.tile([128, 512], F32, tag="pss",
                                    name=f"psf{b}_{i2}_{fo}")
                nc.tensor.matmul(psf, oT[:, i2 * 128:(i2 + 1) * 128],
                                 wf_sb[:, fo * 512:(fo + 1) * 512],
                                 start=True, stop=True)
                # split evacuation between ScalarE and VectorE
                if fo == 0:
                    nc.scalar.copy(yt[:, fo * 512:(fo + 1) * 512], psf)
                else:
                    nc.vector.tensor_copy(yt[:, fo * 512:(fo + 1) * 512], psf)
            nc.sync.dma_start(
                y[b * L + i2 * 128: b * L + (i2 + 1) * 128, :], yt)

        # software-pipelined emission: attention(b) | projections(b+1) | final(b)
        # prologue: only what attention(0,0) needs — kv blocks + first q block
        for blk in range(len(kv_widths)):
            phase1_kv(0, blk)
        phase1_q(0, 0)
        prev = None
        for b in range(B):
            for it in range(4):
                if b == 0 and it + 1 < 4:
                    phase1_q(0, it + 1)
                attn_it(b, it)
                # finals of the PREVIOUS it: their normalize is already done,
                # so these matmuls are ready work that fills this it's tail.
                if prev is not None:
                    pb, pit = prev
                    for i2 in range(pit * 4, pit * 4 + 4):
                        final_yt(pb, i2)
                if b + 1 < B:
                    phase1_block(b + 1, it)
                prev = (b, it)
        for i2 in range(12, 16):
            final_yt(B - 1, i2)

    nc.compile()
    return nc


def make_in_maps(x, mask, Wq, bq, Wk, bk, Wv, bv, Wf, bf, njt):
    bf16 = ml_dtypes.bfloat16
    lc = njt * 128
    xt = np.ascontiguousarray(x.reshape(R, D).T).astype(bf16)

    # compacted key columns + their validity weights
    cols = np.zeros(B * lc, np.int64)
    mprime = np.zeros(B * lc, np.float32)
    m = np.asarray(mask)
    for b in range(B):
        idx = np.nonzero(m[b] == 0)[0] + b * L
        cols[b * lc: b * lc + len(idx)] = idx
        mprime[b * lc: b * lc + len(idx)] = 1.0
    xtc = np.ascontiguousarray(xt[:, cols])
    mp = np.ascontiguousarray(mprime.reshape(B * njt, 128).T).astype(np.float32)

    in_maps = []
    for c in range(NCORES):
        sl = slice(c * DC, (c + 1) * DC)
        wq_c = np.ascontiguousarray((Wq[sl, :] * SCALE).T).astype(bf16).reshape(KCH, 128, DC)
        wk_c = np.ascontiguousarray(Wk[sl, :].T).astype(bf16).reshape(KCH, 128, DC)
        wv_c = np.ascontiguousarray(Wv[sl, :].T).astype(bf16).reshape(KCH, 128, DC)
        wf_c = np.ascontiguousarray(Wf[:, sl].T).astype(bf16)
        bq_c = (bq[sl] * SCALE).astype(np.float32).reshape(128, 1)
        bk_c = bk[sl].astype(np.float32).reshape(128, 1)
        in_maps.append({
            "xt": xt, "xtc": xtc, "wq": wq_c, "wk": wk_c, "wv": wv_c,
            "wf": wf_c, "bq": bq_c, "bk": bk_c, "mp": mp,
        })
    return in_maps


_NC_CACHE = {}


def kernel(**inputs):
    x = np.asarray(inputs["x"], np.float32)
    mask = np.asarray(inputs["mask"])
    Wq = np.asarray(inputs["Wq"], np.float32)
    bq = np.asarray(inputs["bq"], np.float32)
    Wk = np.asarray(inputs["Wk"], np.float32)
    bk = np.asarray(inputs["bk"], np.float32)
    Wv = np.asarray(inputs["Wv"], np.float32)
    bv = np.asarray(inputs["bv"], np.float32)
    Wf = np.asarray(inputs["Wf"], np.float32)
    bf = np.asarray(inputs["bf"], np.float32)

    nmax = int((mask == 0).sum(axis=1).max())
    nmax = max(nmax, 1)
    njt = (nmax + 127) // 128

    in_maps = make_in_maps(x, mask, Wq, bq, Wk, bk, Wv, bv, Wf, bf, njt)
    if njt not in _NC_CACHE:
        _NC_CACHE[njt] = build_bass(njt)
    nc = _NC_CACHE[njt]

    res = run_bass_kernel_spmd(nc, in_maps, core_ids=list(range(NCORES)),
                               trace=TRACE)
    LAST_RESULT["exec_time_ns"] = res.exec_time_ns
    LAST_RESULT["trace"] = res.instructions_and_trace

    y = res.results[0]["y"].astype(np.float32).copy()
    for c in range(1, NCORES):
        y += res.results[c]["y"]
    # bv/bf fold: softmax rows sum to 1, so attn_out bias bv contributes
    # Wf @ bv; bf is the plain output bias.
    y += (Wf @ bv + bf)[None, :]
    return y.reshape(B, L, D).astype(np.float32)


# revision 35
# speedup vs baseline: 1.1052x; 1.0477x over previous
"""Multi-head attention (B=4, L=2048, D=1024, H=16) on 8 Trainium2 NeuronCores.

Sharding: tensor-parallel over heads — core c owns heads {2c, 2c+1}.
Each core computes Q/K/V projections for its 128 feature dims, full
attention for its 2 heads over all 4 batches, and a partial final
projection y_c = attn_out_c @ Wf[:, 128c:128c+128].T.  The host sums the
8 partials (plus the bias constant Wf@bv + bf, which commutes out since
softmax rows sum to 1).

Key compaction: masked keys contribute exactly 0 to softmax(masked
logits), so the host gathers only the unmasked key rows (padded to a
multiple of 128, zero-weighted via m') and K/V projections + QK + exp +
PV run on the compacted key set only (~56% of L for a p=0.5 mask).

Device layouts (per core):
  qT     [128(2h x 64d), L]   bf16   (feature dims in partitions)
  kT     [128, LC]            bf16   (compacted keys)
  v'     [128(j rows), NJT, 130] bf16: per j-row-tile [v_h0*m | m | v_h1*m | m]
         where m = 1 for real keys, 0 for padding (this implements both
         masking and padding: exp(logit)*m / sum_j exp*m)
  logitsT tiles [128 j, 512 i] in PSUM -> exp on ScalarE -> bf16 SBUF
  PV: out'[65, 512] = v'.T @ expT accumulated over NJT j-tiles; row 64 is
      the softmax denominator (ones-column trick).
  normalize: recip_approx(row64), gpsimd partition-broadcast, DVE multiply.
  final: y[i,fo] = outT.T @ WfcT, K=128 (both heads, already normalized).

Emission is software-pipelined across batches (attention(b) interleaved
with projections(b+1) and final(b)) so the PE always has ready matmuls
and the HAM clock gate stays at full rate.
"""

import numpy as np
import ml_dtypes
from contextlib import ExitStack

import concourse.bass as bass
import concourse.tile as tile
from concourse import bacc, mybir
from concourse import masks
from concourse.bass_utils import run_bass_kernel_spmd

B, L, D, H, DK = 4, 2048, 1024, 16, 64
R = B * L                      # 8192 rows
NCORES = 8
HPC = H // NCORES              # 2 heads per core
DC = HPC * DK                  # 128 feature dims per core
KCH = D // 128                 # 8 contraction chunks
SCALE = 1.0 / np.sqrt(DK)

BF16 = mybir.dt.bfloat16
F32 = mybir.dt.float32
EXP = mybir.ActivationFunctionType.Exp

TRACE = False                  # test.py flips this for profiling runs
LAST_RESULT = {}               # exec_time_ns etc. stashed here for test.py


def build_bass(njts):
    """njts = per-batch number of 128-key tiles after compaction."""
    lcs = [n * 128 for n in njts]
    lc_off = [sum(lcs[:b]) for b in range(B)]       # xtc column offsets
    jt_off = [sum(njts[:b]) for b in range(B)]      # mp column offsets

    def widths(b):
        lc = lcs[b]
        return [512] * (lc // 512) + ([lc % 512] if lc % 512 else [])

    nc = bacc.Bacc(
        "TRN2",
        target_bir_lowering=False,
        debug=False,
        enable_asserts=False,
        num_devices=NCORES,
    )
    xt = nc.dram_tensor("xt", [D, R], BF16, kind="ExternalInput").ap()
    xtc = nc.dram_tensor("xtc", [D, sum(lcs)], BF16, kind="ExternalInput").ap()
    wq = nc.dram_tensor("wq", [KCH, 128, DC], BF16, kind="ExternalInput").ap()
    wk = nc.dram_tensor("wk", [KCH, 128, DC], BF16, kind="ExternalInput").ap()
    wv = nc.dram_tensor("wv", [KCH, 128, DC], BF16, kind="ExternalInput").ap()
    wf = nc.dram_tensor("wf", [DC, D], BF16, kind="ExternalInput").ap()
    bq = nc.dram_tensor("bq", [128, 1], F32, kind="ExternalInput").ap()
    bk = nc.dram_tensor("bk", [128, 1], F32, kind="ExternalInput").ap()
    mp = nc.dram_tensor("mp", [128, sum(njts)], F32, kind="ExternalInput").ap()
    y = nc.dram_tensor("y", [R, D], F32, kind="ExternalOutput").ap()

    with tile.TileContext(nc) as tc, ExitStack() as ctx:
        singles = ctx.enter_context(tc.tile_pool(name="singles", bufs=1))
        qkpool = ctx.enter_context(tc.tile_pool(name="qk", bufs=3))
        xin = ctx.enter_context(tc.tile_pool(name="xin", bufs=6))
        expp = ctx.enter_context(tc.tile_pool(name="expp", bufs=6))
        pvsb = ctx.enter_context(tc.tile_pool(name="pvsb", bufs=3))
        ysb = ctx.enter_context(tc.tile_pool(name="ysb", bufs=3))
        ps_small = ctx.enter_context(tc.tile_pool(name="ps_small", bufs=2, space="PSUM"))
        ps_pair = ctx.enter_context(tc.tile_pool(name="ps_pair", bufs=2, space="PSUM"))
        ps_pv = ctx.enter_context(tc.tile_pool(name="ps_pv", bufs=2, space="PSUM"))

        wq_sb = singles.tile([128, KCH, DC], BF16)
        nc.sync.dma_start(wq_sb, wq.rearrange("c p f -> p c f"))
        wk_sb = singles.tile([128, KCH, DC], BF16)
        nc.sync.dma_start(wk_sb, wk.rearrange("c p f -> p c f"))
        wv_sb = singles.tile([128, KCH, DC], BF16)
        nc.sync.dma_start(wv_sb, wv.rearrange("c p f -> p c f"))
        wf_sb = singles.tile([DC, D], BF16)
        nc.sync.dma_start(wf_sb, wf)
        bq_sb = singles.tile([128, 1], F32)
        nc.sync.dma_start(bq_sb, bq)
        bk_sb = singles.tile([128, 1], F32)
        nc.sync.dma_start(bk_sb, bk)
        mp_sb = singles.tile([128, sum(njts)], F32)
        nc.sync.dma_start(mp_sb, mp)
        ident_sb = singles.tile([128, 128], BF16)
        masks.make_identity(nc, ident_sb)

        xt_r = xt.rearrange("(c p) r -> p c r", p=128)
        xtc_r = xtc.rearrange("(c p) r -> p c r", p=128)

        bt = {}  # per-batch persistent tiles

        def tiles(b):
            if b not in bt:
                bt[b] = (
                    qkpool.tile([128, L], BF16, tag="qT", name=f"qT{b}"),
                    qkpool.tile([128, lcs[b]], BF16, tag="kT", name=f"kT{b}"),
                    qkpool.tile([128, njts[b], 2 * (DK + 1)], BF16, tag="vb",
                                name=f"vb{b}"),
                    qkpool.tile([128, L], BF16, tag="oT", name=f"oT{b}"),
                )
            return bt[b]

        def phase1_q(b, blk):
            """Q projection for rows [b*L + blk*512, +512)."""
            qT, _, _, _ = tiles(b)
            r0 = b * L + blk * 512
            xt_t = xin.tile([128, KCH, 512], BF16, tag="xt", name=f"xt{b}_{blk}")
            nc.sync.dma_start(xt_t, xt_r[:, :, r0:r0 + 512])
            ps = ps_small.tile([128, 512], F32, tag="pss", name=f"psq{b}_{blk}")
            for k in range(KCH):
                nc.tensor.matmul(ps, wq_sb[:, k, :], xt_t[:, k, :],
                                 start=(k == 0), stop=(k == KCH - 1))
            nc.vector.tensor_scalar_add(
                qT[:, blk * 512:(blk + 1) * 512], ps, bq_sb)

        def phase1_kv(b, blk):
            """K + V projections for compacted keys [blk*512, +w) of batch b."""
            _, kT, vb, _ = tiles(b)
            w = widths(b)[blk]
            c0 = blk * 512
            xc_t = xin.tile([128, KCH, 512], BF16, tag="xc", name=f"xc{b}_{blk}")
            nc.sync.dma_start(xc_t[:, :, 0:w],
                              xtc_r[:, :, lc_off[b] + c0: lc_off[b] + c0 + w])
            ps = ps_small.tile([128, 512], F32, tag="pss", name=f"psk{b}_{blk}")
            for k in range(KCH):
                nc.tensor.matmul(ps[:, 0:w], wk_sb[:, k, :], xc_t[:, k, 0:w],
                                 start=(k == 0), stop=(k == KCH - 1))
            nc.vector.tensor_scalar_add(kT[:, c0:c0 + w], ps[:, 0:w], bk_sb)
            # vT block [d 128, r w] then PE-transpose into [r, d] v' tiles
            psvT = ps_small.tile([128, 512], F32, tag="pss", name=f"psvT{b}_{blk}")
            for k in range(KCH):
                nc.tensor.matmul(psvT[:, 0:w], wv_sb[:, k, :], xc_t[:, k, 0:w],
                                 start=(k == 0), stop=(k == KCH - 1))
            vt_sb = xin.tile([128, 512], BF16, tag="vt", name=f"vt{b}_{blk}")
            nc.vector.tensor_copy(vt_sb[:, 0:w], psvT[:, 0:w])
            for rs in range(w // 128):
                t = blk * 4 + rs            # j-tile within batch
                tg = jt_off[b] + t          # global compacted row-tile
                pst = ps_small.tile([128, 128], BF16, tag="pss",
                                    name=f"pst{b}_{blk}_{rs}")
                nc.tensor.transpose(pst, vt_sb[:, rs * 128:(rs + 1) * 128],
                                    ident_sb)
                mcol = mp_sb[:, tg:tg + 1]
                nc.vector.tensor_scalar_mul(vb[:, t, 0:DK], pst[:, 0:DK], mcol)
                nc.vector.tensor_scalar_mul(
                    vb[:, t, DK + 1:2 * DK + 1], pst[:, DK:DC], mcol)
                nc.vector.tensor_copy(vb[:, t, DK:DK + 1], mcol)
                nc.vector.tensor_copy(vb[:, t, 2 * DK + 1:2 * DK + 2], mcol)

        def phase1_block(b, blk):
            phase1_q(b, blk)
            if blk < len(widths(b)):
                phase1_kv(b, blk)

        def attn_it(b, it, fillers=()):
            """Attention for queries [b*L + it*512, +512), both heads."""
            fillers = list(fillers)
            qT, kT, vb, oT = tiles(b)
            i0 = it * 512
            njt = njts[b]
            pvs = [ps_pv.tile([DK + 1, 512], F32, tag="pv", name=f"pv{b}_{it}_{h}")
                   for h in range(HPC)]
            for jp in range((njt + 1) // 2):
                if fillers:
                    fillers.pop(0)()
                ns = min(2, njt - jp * 2)
                for h in range(HPC):
                    ps = ps_pair.tile([128, 2, 512], F32, tag="pair",
                                      name=f"qk{b}_{it}_{jp}_{h}")
                    for s in range(ns):
                        jt = jp * 2 + s
                        nc.tensor.matmul(
                            ps[:, s, :],
                            kT[h * DK:(h + 1) * DK, jt * 128:(jt + 1) * 128],
                            qT[h * DK:(h + 1) * DK, i0:i0 + 512],
                            start=True, stop=True,
                            tile_position=(h * DK, 0))
                    ex = expp.tile([128, 2, 512], BF16, tag="ex",
                                   name=f"ex{b}_{it}_{jp}_{h}")
                    nc.scalar.activation(ex[:, 0:ns, :], ps[:, 0:ns, :], EXP)
                    for s in range(ns):
                        jt = jp * 2 + s
                        nc.tensor.matmul(
                            pvs[h],
                            vb[:, jt, h * (DK + 1):(h + 1) * (DK + 1)],
                            ex[:, s, :],
                            start=(jt == 0), stop=(jt == njt - 1))
            # normalize: rrow = 1/sums (both heads batched in one partition),
            # broadcast via gpsimd, multiply.
            srow = pvsb.tile([1, HPC, 512], F32, tag="srow", name=f"sr{b}_{it}")
            for h in range(HPC):
                nc.vector.tensor_copy(srow[0:1, h, :], pvs[h][DK:DK + 1, :])
            rrow = pvsb.tile([1, HPC, 512], F32, tag="rrow", name=f"rr{b}_{it}")
            nc.vector.reciprocal_approx_fast(rrow, srow)
            for h in range(HPC):
                rb_sb = pvsb.tile([DK, 512], F32, tag="rb", name=f"rb{b}_{it}_{h}")
                nc.gpsimd.partition_broadcast(rb_sb, rrow[0:1, h, :])
                nc.vector.tensor_mul(
                    oT[h * DK:(h + 1) * DK, i0:i0 + 512], pvs[h][0:DK, :], rb_sb)

        def final_yt(b, i2):
            """Final projection for one 128-row i-tile of batch b."""
            _, _, _, oT = tiles(b)
            yt = ysb.tile([128, D], F32, tag="y", name=f"y{b}_{i2}")
            for fo in range(D // 512):
                psf = ps_small.tile([128, 512], F32, tag="pss",
                                    name=f"psf{b}_{i2}_{fo}")
                nc.tensor.matmul(psf, oT[:, i2 * 128:(i2 + 1) * 128],
                                 wf_sb[:, fo * 512:(fo + 1) * 512],
                                 start=True, stop=True)
                # split evacuation between ScalarE and VectorE
                if fo == 0:
                    nc.scalar.copy(yt[:, fo * 512:(fo + 1) * 512], psf)
                else:
                    nc.vector.tensor_copy(yt[:, fo * 512:(fo + 1) * 512], psf)
            nc.sync.dma_start(
                y[b * L + i2 * 128: b * L + (i2 + 1) * 128, :], yt)

        # software-pipelined emission: attention(b) | projections(b+1) | final(b)
        # prologue: only what attention(0,0) needs — kv blocks + first q block
        for blk in range(len(widths(0))):
            phase1_kv(0, blk)
        phase1_q(0, 0)
        prev = None
        for b in range(B):
            for it in range(4):
                if b == 0 and it + 1 < 4:
                    phase1_q(0, it + 1)
                attn_it(b, it)
                # finals of the PREVIOUS it: their normalize is already done,
                # so these matmuls are ready work that fills this it's tail.
                if prev is not None:
                    pb, pit = prev
                    for i2 in range(pit * 4, pit * 4 + 4):
                        final_yt(pb, i2)
                if b + 1 < B:
                    phase1_block(b + 1, it)
                prev = (b, it)
        for i2 in range(12, 16):
            final_yt(B - 1, i2)

    nc.compile()
    return nc


def make_in_maps(x, mask, Wq, bq, Wk, bk, Wv, bv, Wf, bf, njts):
    bf16 = ml_dtypes.bfloat16
    lcs = [n * 128 for n in njts]
    xt = np.ascontiguousarray(x.reshape(R, D).T).astype(bf16)

    # compacted key columns + their validity weights
    cols = np.zeros(sum(lcs), np.int64)
    mprime = np.zeros(sum(lcs), np.float32)
    m = np.asarray(mask)
    off = 0
    for b in range(B):
        idx = np.nonzero(m[b] == 0)[0] + b * L
        cols[off: off + len(idx)] = idx
        mprime[off: off + len(idx)] = 1.0
        off += lcs[b]
    xtc = np.ascontiguousarray(xt[:, cols])
    mp = np.ascontiguousarray(mprime.reshape(sum(njts), 128).T).astype(np.float32)

    in_maps = []
    for c in range(NCORES):
        sl = slice(c * DC, (c + 1) * DC)
        wq_c = np.ascontiguousarray((Wq[sl, :] * SCALE).T).astype(bf16).reshape(KCH, 128, DC)
        wk_c = np.ascontiguousarray(Wk[sl, :].T).astype(bf16).reshape(KCH, 128, DC)
        wv_c = np.ascontiguousarray(Wv[sl, :].T).astype(bf16).reshape(KCH, 128, DC)
        wf_c = np.ascontiguousarray(Wf[:, sl].T).astype(bf16)
        bq_c = (bq[sl] * SCALE).astype(np.float32).reshape(128, 1)
        bk_c = bk[sl].astype(np.float32).reshape(128, 1)
        in_maps.append({
            "xt": xt, "xtc": xtc, "wq": wq_c, "wk": wk_c, "wv": wv_c,
            "wf": wf_c, "bq": bq_c, "bk": bk_c, "mp": mp,
        })
    return in_maps


_NC_CACHE = {}


def kernel(**inputs):
    x = np.asarray(inputs["x"], np.float32)
    mask = np.asarray(inputs["mask"])
    Wq = np.asarray(inputs["Wq"], np.float32)
    bq = np.asarray(inputs["bq"], np.float32)
    Wk = np.asarray(inputs["Wk"], np.float32)
    bk = np.asarray(inputs["bk"], np.float32)
    Wv = np.asarray(inputs["Wv"], np.float32)
    bv = np.asarray(inputs["bv"], np.float32)
    Wf = np.asarray(inputs["Wf"], np.float32)
    bf = np.asarray(inputs["bf"], np.float32)

    nb = (np.asarray(mask) == 0).sum(axis=1)
    njts = tuple(max(1, int(n + 127) // 128) for n in nb)

    in_maps = make_in_maps(x, mask, Wq, bq, Wk, bk, Wv, bv, Wf, bf, njts)
    if njts not in _NC_CACHE:
        _NC_CACHE[njts] = build_bass(njts)
    nc = _NC_CACHE[njts]

    res = run_bass_kernel_spmd(nc, in_maps, core_ids=list(range(NCORES)),
                               trace=TRACE)
    LAST_RESULT["exec_time_ns"] = res.exec_time_ns
    LAST_RESULT["trace"] = res.instructions_and_trace

    y = res.results[0]["y"].astype(np.float32).copy()
    for c in range(1, NCORES):
        y += res.results[c]["y"]
    # bv/bf fold: softmax rows sum to 1, so attn_out bias bv contributes
    # Wf @ bv; bf is the plain output bias.
    y += (Wf @ bv + bf)[None, :]
    return y.reshape(B, L, D).astype(np.float32)
